# revision 27
# baseline (speedup 1.0000x reference)
"""Trainium2 Bass kernel for nn_NeuralNetwork_5274219839793.

DEQ-style model: z0 = GRU_x(x); z* = fixpoint of f(z) = tanh(GRU_z(z) + z0);
out = (one more f step).reshape(B,-1) @ Wout.T + bout.

Strategy
--------
Data parallel over batch: 8 NeuronCores x 8 batch rows each, weights
replicated, no collectives.

Instead of stepping the GRU recurrences sequentially over T=128 (which is
latency-bound on Trainium), the whole hidden *trajectory* is treated as the
fixed-point variable (the problem is already a fixed-point solve, so this
changes nothing about the answer): each sweep computes all gate
pre-activations for all timesteps with large matmuls using the *previous*
sweep's trajectory (Jacobi lag), then runs the exact affine recurrence
h_t = zg_t * h_{t-1} + (1-zg_t) * n_t with the hardware prefix-scan
(tensor_tensor_scan), fully parallel over t.  The lagged iteration has the
same fixed point; measured convergence: x-GRU trajectory exact to ~2e-7
absmax in 14 sweeps; z-phase reaches the reference's own convergence floor
(rel err ~4e-5, the reference's Anderson loop stops with residual ~4e-5)
in ~40 sweeps.  The final answer is read directly from the sweep state
Z = tanh(H + z0).

Per-core layouts (partition dim first):
  xT      [64, (b,t)=1024]           input, transposed on PE
  gi_x    [128, m=6, (b,t)=1024]     Wih_x @ x^T, precomputed once
  Hx/Hz   [128, k=2, b=8, t=129]     hidden trajectories; t=0 slot == 0
  Z       [128, k=2, b=8, t=128]     current z iterate (= tanh(Hz+z0))
  weights WT [128, k=2, 768]         pre-transposed on host
"""

import numpy as np

B, T, D, H = 64, 128, 64, 256
NCORES = 8
BC = B // NCORES          # batch rows per core
NSX = 16                  # x-GRU sweeps
NSZ = 44                  # z fixed-point sweeps
BT = BC * T               # 1024 moving columns
F32 = None                # set lazily (mybir.dt.float32)

_cache = {}


def _blob_offsets():
    """Column offsets of the packed [128, BLOB_COLS] per-core input blob.
    One single DMA keeps every instruction's semaphore-wait fan-in tiny."""
    names = [
        ("whhxT", 1536), ("wihzT", 1536), ("whhzT", 1536), ("woutT", 256),
        ("ident", 128), ("brz_x", 4), ("bhhn_x", 2), ("nbihn_x", 2),
        ("brz_z", 4), ("bhhn_z", 2), ("nbihn_z", 2), ("ones", 1), ("bout", 1),
        ("wihxT", 3 * H), ("x", (B // NCORES) * T * D // 128),
    ]
    off, out = 0, {}
    for k, w in names:
        out[k] = off
        off += w
    out["_total"] = off
    return out


BLOB_COLS = _blob_offsets()["_total"]


def _build_program():
    import concourse.bass as bass
    import concourse.bacc as bacc
    import concourse.mybir as mybir
    import concourse.tile as tile
    from concourse.bass import ts

    F32 = mybir.dt.float32
    AF = mybir.ActivationFunctionType
    OP = mybir.AluOpType

    nc = bacc.Bacc()

    # ---- DRAM I/O ----
    # Everything (weights + biases + identity + this core's x shard) is
    # host-packed into ONE [128, BLOB_COLS] blob loaded with ONE DMA, so
    # downstream instructions wait on at most {1 DMA queue, 1-2 engines}.
    blob_d = nc.dram_tensor("blob", [128, BLOB_COLS], F32, kind="ExternalInput")
    out_d = nc.dram_tensor("out", [1, BC], F32, kind="ExternalOutput")

    from contextlib import ExitStack

    with tile.TileContext(nc) as tc, ExitStack() as ctx:
        consts = ctx.enter_context(tc.tile_pool(name="consts", bufs=1))
        state = ctx.enter_context(tc.tile_pool(name="state", bufs=1))
        work = ctx.enter_context(tc.tile_pool(name="work", bufs=3))
        scanw = ctx.enter_context(tc.tile_pool(name="scanw", bufs=2))
        psrz = ctx.enter_context(tc.tile_pool(name="psrz", bufs=2, space="PSUM"))
        psn = ctx.enter_context(tc.tile_pool(name="psn", bufs=1, space="PSUM"))
        psgi = ctx.enter_context(tc.tile_pool(name="psgi", bufs=1, space="PSUM"))

        # ---- load constants (ONE DMA) ----
        blob = consts.tile([128, BLOB_COLS], F32)
        nc.sync.dma_start(out=blob, in_=blob_d[:, :])

        o = _blob_offsets()
        wihxT = blob[0:64, o["wihxT"] : o["wihxT"] + 3 * H]
        xall = blob[:, o["x"] : o["x"] + BT * D // 128].rearrange(
            "p (c d) -> p c d", c=BT // 128)
        whhxT = blob[:, o["whhxT"] : o["whhxT"] + 1536].rearrange(
            "p (k m) -> p k m", k=2)
        wihzT = blob[:, o["wihzT"] : o["wihzT"] + 1536].rearrange(
            "p (k m) -> p k m", k=2)
        whhzT = blob[:, o["whhzT"] : o["whhzT"] + 1536].rearrange(
            "p (k m) -> p k m", k=2)
        woutT = blob[:, o["woutT"] : o["woutT"] + 256].rearrange(
            "p (k t) -> p k t", k=2)
        ident = blob[:, o["ident"] : o["ident"] + 128]
        brz_x = blob[:, o["brz_x"] : o["brz_x"] + 4]
        bhhn_x = blob[:, o["bhhn_x"] : o["bhhn_x"] + 2]
        nbihn_x = blob[:, o["nbihn_x"] : o["nbihn_x"] + 2]
        brz_z = blob[:, o["brz_z"] : o["brz_z"] + 4]
        bhhn_z = blob[:, o["bhhn_z"] : o["bhhn_z"] + 2]
        nbihn_z = blob[:, o["nbihn_z"] : o["nbihn_z"] + 2]
        ones = blob[:, o["ones"] : o["ones"] + 1]
        boutt = blob[0:1, o["bout"] : o["bout"] + 1]

        # ---- persistent state ----
        xT = state.tile([64, BT], F32)
        gi_x = state.tile([128, 6, BT], F32)
        Hx = state.tile([128, 2, BC, T + 1], F32)
        Hz = state.tile([128, 2, BC, T + 1], F32)
        Z = state.tile([128, 2, BC, T], F32)
        rsb = state.tile([128, 2, BT], F32)
        zgsb = state.tile([128, 2, BT], F32)

        nc.vector.memset(Hx, 0.0)
        nc.vector.memset(Hz, 0.0)

        # ---- transpose x: [BT, 64] -> xT [64, BT] ----
        for ch in range(BT // 128):
            pst = psrz.tile([64, 128], F32, tag="prz")
            nc.tensor.transpose(out=pst, in_=xall[:, ch, :], identity=ident)
            nc.vector.tensor_copy(out=xT[:, ts(ch, 128)], in_=pst)

        # ---- gi_x = Wih_x @ x^T (once) ----
        for m in range(6):
            pgi = psgi.tile([128, BT], F32, tag="pgi")
            for nch in range(2):
                nc.tensor.matmul(
                    pgi[:, ts(nch, 512)],
                    wihxT[:, ts(m, 128)],
                    xT[:, ts(nch, 512)],
                    start=True, stop=True,
                )
            nc.vector.tensor_copy(out=gi_x[:, m, :], in_=pgi)

        # ---- sweep body (shared between x-phase and z-phase) ----
        def sweep(s, Hbuf, whhT, brz, bhhn, nbihn, phase):
            """One Jacobi sweep.  phase: 'x' (gi from gi_x SBUF via identity
            matmul) or 'z' (gi matmuls from Z; on s==0 Z is read from Hx)."""
            first = s == 0

            def z_rhs(k, nch):
                # batch group of 4 (=512 moving columns)
                if phase == "z" and first:
                    return Hx[:, k, ts(nch, 4), 1 : T + 1]
                return Z[:, k, ts(nch, 4), :]

            # r and zg gates: m-tiles 0..3, PSUM accumulates gi + gh
            for m in range(4):
                prz = psrz.tile([128, BT], F32, tag="prz")
                for nch in range(2):
                    ops = []
                    if phase == "z":
                        ops += [("wih", k) for k in range(2)]
                    else:
                        ops += [("gi", 0)]
                    if not first:
                        ops += [("whh", k) for k in range(2)]
                    for i, (kind, k) in enumerate(ops):
                        st, sp = i == 0, i == len(ops) - 1
                        if kind == "wih":
                            nc.tensor.matmul(
                                prz[:, ts(nch, 512)],
                                wihzT[:, k, ts(m, 128)],
                                z_rhs(k, nch),
                                start=st, stop=sp,
                            )
                        elif kind == "gi":
                            nc.tensor.matmul(
                                prz[:, ts(nch, 512)],
                                ident,
                                gi_x[:, m, ts(nch, 512)],
                                start=st, stop=sp,
                            )
                        else:
                            nc.tensor.matmul(
                                prz[:, ts(nch, 512)],
                                whhT[:, k, ts(m, 128)],
                                Hbuf[:, k, ts(nch, 4), 0:T],
                                start=st, stop=sp,
                            )
                dest = rsb if m < 2 else zgsb
                nc.scalar.activation(
                    out=dest[:, m % 2, :], in_=prz,
                    func=AF.Sigmoid, bias=brz[:, m : m + 1], scale=1.0,
                )

            # n gate per k-tile (m-tiles 4,5).  All matmuls/gate math for BOTH
            # k-tiles complete before any scan runs: the scans overwrite Hbuf,
            # which every gate matmul in this sweep must read at its OLD
            # (previous-sweep) value — pure Jacobi, as validated numerically.
            d1 = scanw.tile([128, 2, BT], F32, tag="d1")
            for km in range(2):
                m = 4 + km
                if phase == "z":
                    pg = psgi.tile([128, BT], F32, tag="pgi")
                    for nch in range(2):
                        for k in range(2):
                            nc.tensor.matmul(
                                pg[:, ts(nch, 512)],
                                wihzT[:, k, ts(m, 128)],
                                z_rhs(k, nch),
                                start=k == 0, stop=k == 1,
                            )
                    gi_n = pg
                else:
                    gi_n = gi_x[:, m, :]

                tmp = work.tile([128, BT], F32, tag="tmp")
                if first:
                    nc.vector.tensor_scalar_mul(
                        out=tmp, in0=rsb[:, km, :], scalar1=bhhn[:, km : km + 1]
                    )
                else:
                    pn = psn.tile([128, BT], F32, tag="pn")
                    for nch in range(2):
                        for k in range(2):
                            nc.tensor.matmul(
                                pn[:, ts(nch, 512)],
                                whhT[:, k, ts(m, 128)],
                                Hbuf[:, k, ts(nch, 4), 0:T],
                                start=k == 0, stop=k == 1,
                            )
                    nc.vector.scalar_tensor_tensor(
                        out=tmp, in0=pn, scalar=bhhn[:, km : km + 1],
                        in1=rsb[:, km, :], op0=OP.add, op1=OP.mult,
                    )
                npre = work.tile([128, BT], F32, tag="npre")
                nc.vector.tensor_add(out=npre, in0=tmp, in1=gi_n)
                negn = work.tile([128, BT], F32, tag="negn")
                nc.scalar.activation(
                    out=negn, in_=npre, func=AF.Tanh,
                    bias=nbihn[:, km : km + 1], scale=-1.0,
                )
                nc.vector.scalar_tensor_tensor(
                    out=d1[:, km, :], in0=zgsb[:, km, :], scalar=1.0, in1=negn,
                    op0=OP.subtract, op1=OP.mult,
                )
            for km in range(2):
                for b in range(BC):
                    nc.vector.tensor_tensor_scan(
                        out=Hbuf[:, km, b, 1 : T + 1],
                        data0=zgsb[:, km, ts(b, T)],
                        data1=d1[:, km, ts(b, T)],
                        initial=0.0, op0=OP.mult, op1=OP.add,
                    )

            if phase == "z":
                for km in range(2):
                    zpre = work.tile([128, BC, T], F32, tag="zpre")
                    nc.vector.tensor_add(
                        out=zpre,
                        in0=Hbuf[:, km, :, 1 : T + 1],
                        in1=Hx[:, km, :, 1 : T + 1],
                    )
                    nc.scalar.activation(
                        out=Z[:, km, :, :], in_=zpre, func=AF.Tanh,
                    )

        for s in range(NSX):
            sweep(s, Hx, whhxT, brz_x, bhhn_x, nbihn_x, "x")
        for s in range(NSZ):
            sweep(s, Hz, whhzT, brz_z, bhhn_z, nbihn_z, "z")

        # ---- head: out[b] = sum_(p,k,t) Z * woutT + bout ----
        red = work.tile([128, 2, BC], F32, tag="red")
        for k in range(2):
            prod = work.tile([128, BC, T], F32, tag="prod")
            wslice = woutT[:, k, :]
            wbcast = bass.AP(
                tensor=wslice.tensor,
                offset=wslice.offset,
                ap=[wslice.ap[0], [0, BC], wslice.ap[1]],
            )
            nc.vector.tensor_mul(out=prod, in0=Z[:, k, :, :], in1=wbcast)
            nc.vector.reduce_sum(
                out=red[:, k, :], in_=prod, axis=mybir.AxisListType.X
            )
        redk = work.tile([128, BC], F32, tag="redk")
        nc.vector.tensor_add(out=redk, in0=red[:, 0, :], in1=red[:, 1, :])
        ph = psn.tile([1, BC], F32, tag="pn")
        nc.tensor.matmul(ph, ones, redk, start=True, stop=True)
        osb = work.tile([1, BC], F32, tag="osb")
        nc.vector.tensor_scalar_add(out=osb, in0=ph, scalar1=boutt[0:1, 0:1])
        nc.sync.dma_start(out=out_d[:, :], in_=osb)

    nc.compile()
    return nc


def _prep_shared(inputs):
    """Host-side weight preprocessing (shared across cores)."""
    f = np.float32
    Wih_x, Whh_x = inputs["Wih_x"], inputs["Whh_x"]
    Wih_z, Whh_z = inputs["Wih_z"], inputs["Whh_z"]
    bih_x, bhh_x = inputs["bih_x"], inputs["bhh_x"]
    bih_z, bhh_z = inputs["bih_z"], inputs["bhh_z"]

    def biases(bih, bhh):
        brz = (bih[: 2 * H] + bhh[: 2 * H]).astype(f)          # [512]
        brz = brz.reshape(4, 128).T.copy()                     # [128, 4]
        bhhn = bhh[2 * H :].astype(f).reshape(2, 128).T.copy() # [128, 2]
        nbihn = (-bih[2 * H :]).astype(f).reshape(2, 128).T.copy()
        return brz, bhhn, nbihn

    brz_x, bhhn_x, nbihn_x = biases(bih_x, bhh_x)
    brz_z, bhhn_z, nbihn_z = biases(bih_z, bhh_z)
    woutT = (
        inputs["Wout"].astype(f).reshape(T, 2, 128).transpose(2, 1, 0).copy()
    )  # [128, 2, T]

    def packT(w):  # [256, 768] -> [128, (k,m)=1536]
        return np.ascontiguousarray(
            w.T.astype(f).reshape(2, 128, 3 * H).transpose(1, 0, 2).reshape(128, -1)
        )

    o = _blob_offsets()
    blob = np.zeros((128, BLOB_COLS), dtype=f)

    def put(name, arr):
        arr = np.asarray(arr, dtype=f)
        blob[: arr.shape[0], o[name] : o[name] + arr.shape[1]] = arr

    put("whhxT", packT(Whh_x))
    put("wihzT", packT(Wih_z))
    put("whhzT", packT(Whh_z))
    put("woutT", woutT.reshape(128, 256))
    put("ident", np.eye(128, dtype=f))
    put("brz_x", brz_x)
    put("bhhn_x", bhhn_x)
    put("nbihn_x", nbihn_x)
    put("brz_z", brz_z)
    put("bhhn_z", bhhn_z)
    put("nbihn_z", nbihn_z)
    put("ones", np.ones((128, 1), dtype=f))
    put("bout", inputs["bout"].astype(f).reshape(1, 1))
    put("wihxT", Wih_x.T.astype(f))
    return blob


def kernel(**inputs):
    from concourse.bass_utils import run_bass_kernel_spmd

    if "nc" not in _cache:
        _cache["nc"] = _build_program()
    nc = _cache["nc"]

    blob = _prep_shared(inputs)
    o = _blob_offsets()
    x = inputs["x"].astype(np.float32)
    in_maps = []
    for c in range(NCORES):
        cb = blob.copy()
        # x shard packed as [128, (chunk, d)]: row (b,t) = chunk*128 + p
        xs = (
            x[c * BC : (c + 1) * BC]
            .reshape(BT // 128, 128, D)
            .transpose(1, 0, 2)
            .reshape(128, -1)
        )
        cb[:, o["x"] : o["x"] + xs.shape[1]] = xs
        in_maps.append({"blob": cb})

    import contextlib
    import os

    prof_dir = os.environ.get("KERNEL_PROFILE_DIR")
    prof_ctx = contextlib.nullcontext()
    if prof_dir:
        try:
            from antenv.axon_hooks import get_axon_ntff_profile_hook

            hook = get_axon_ntff_profile_hook()
            if hook is not None:
                prof_ctx = hook(prof_dir, None)
        except ImportError:
            pass
    with prof_ctx:
        res = run_bass_kernel_spmd(nc, in_maps, core_ids=list(range(NCORES)))
    kernel._last_results = res
    out = np.empty((B, 1), dtype=np.float32)
    for c in range(NCORES):
        out[c * BC : (c + 1) * BC, 0] = res.results[c]["out"][0]
    return out


# revision 29
# speedup vs baseline: 1.9111x; 1.9111x over previous
"""Trainium2 Bass kernel for nn_NeuralNetwork_5274219839793.

DEQ-style model: z0 = GRU_x(x); z* = fixpoint of f(z) = tanh(GRU_z(z) + z0);
out = f(z*).reshape(B,-1) @ Wout.T + bout.

Strategy
--------
* Data parallel over batch: 8 NeuronCores x 8 batch rows, weights replicated,
  no collectives.
* The whole hidden trajectory is the fixed-point variable: each sweep
  computes all T=128 timesteps' gate pre-activations with large matmuls from
  the previous sweep's trajectory (Jacobi lag), then runs the exact affine
  recurrence h_t = zg_t*h_{t-1} + (1-zg_t)*n_t with the hardware prefix scan
  (tensor_tensor_scan).  Same fixed point, fully parallel over t.
* Mixed precision: fp32 matmuls cost 2 PE passes on trn2, so most sweeps run
  in fp16 (1 pass, full-speed weight loads); the last few sweeps run in fp32
  to polish to the reference's own convergence floor.  CPU-validated:
  (14 fp16 + 2 fp32) x-sweeps and (38 fp16 + 6 fp32) z-sweeps give max rel
  err ~5e-5 vs the reference (which itself stops at residual ~4e-5).
"""

import numpy as np

B, T, D, H = 64, 128, 64, 256
NCORES = 8
BC = B // NCORES
BT = BC * T               # 1024 moving columns per (k-tile)
NSX_LO, NSX_HI = 14, 2    # x-GRU sweeps (fp16, fp32)
NSZ_LO, NSZ_HI = 38, 6    # z fixed-point sweeps (fp16, fp32)

_cache = {}


def _blob_offsets():
    """fp32 blob columns (ONE DMA for all fp32 consts + this core's x)."""
    names = [
        ("whhxT", 1536), ("wihzT", 1536), ("whhzT", 1536), ("woutT", 256),
        ("ident", 128), ("brz_x", 4), ("bhhn_x", 2), ("nbihn_x", 2),
        ("brz_z", 4), ("bhhn_z", 2), ("nbihn_z", 2), ("ones", 1), ("bout", 1),
        ("wihxT", 3 * H), ("x", BC * T * D // 128),
    ]
    off, out = 0, {}
    for k, w in names:
        out[k] = off
        off += w
    out["_total"] = off
    return out


def _blob16_offsets():
    """fp16 blob columns (one more DMA: fp16 weight copies)."""
    names = [
        ("whhxT", 1536), ("wihzT", 1536), ("whhzT", 1536),
        ("ident", 128), ("wihxT", 3 * H),
    ]
    off, out = 0, {}
    for k, w in names:
        out[k] = off
        off += w
    out["_total"] = off
    return out


BLOB_COLS = _blob_offsets()["_total"]
BLOB16_COLS = _blob16_offsets()["_total"]


def _build_program():
    import concourse.bacc as bacc
    import concourse.bass as bass
    import concourse.mybir as mybir
    import concourse.tile as tile
    from concourse.bass import ts
    from contextlib import ExitStack

    F32 = mybir.dt.float32
    F16 = mybir.dt.float16
    AF = mybir.ActivationFunctionType
    OP = mybir.AluOpType

    nc = bacc.Bacc()

    blob_d = nc.dram_tensor("blob", [128, BLOB_COLS], F32, kind="ExternalInput")
    blob16_d = nc.dram_tensor(
        "blob16", [128, BLOB16_COLS], F16, kind="ExternalInput")
    out_d = nc.dram_tensor("out", [1, BC], F32, kind="ExternalOutput")

    with tile.TileContext(nc) as tc, ExitStack() as ctx:
        consts = ctx.enter_context(tc.tile_pool(name="consts", bufs=1))
        state = ctx.enter_context(tc.tile_pool(name="state", bufs=1))
        work = ctx.enter_context(tc.tile_pool(name="work", bufs=2))
        scanw = ctx.enter_context(tc.tile_pool(name="scanw", bufs=1))
        psrz = ctx.enter_context(tc.tile_pool(name="psrz", bufs=2, space="PSUM"))
        psn = ctx.enter_context(tc.tile_pool(name="psn", bufs=1, space="PSUM"))
        psgi = ctx.enter_context(tc.tile_pool(name="psgi", bufs=1, space="PSUM"))

        # ---- constants (2 DMAs) ----
        blob = consts.tile([128, BLOB_COLS], F32)
        nc.sync.dma_start(out=blob, in_=blob_d[:, :])
        blob16 = consts.tile([128, BLOB16_COLS], F16)
        nc.sync.dma_start(out=blob16, in_=blob16_d[:, :])

        o = _blob_offsets()
        o16 = _blob16_offsets()

        def view(base, offs, name, cols, rearr=None, **kw):
            ap = base[:, offs[name] : offs[name] + cols]
            return ap.rearrange(rearr, **kw) if rearr else ap

        wihxT = blob[0:64, o["wihxT"] : o["wihxT"] + 3 * H]
        xall = view(blob, o, "x", BT * D // 128, "p (c d) -> p c d", c=BT // 128)
        whhxT = view(blob, o, "whhxT", 1536, "p (k m) -> p k m", k=2)
        wihzT = view(blob, o, "wihzT", 1536, "p (k m) -> p k m", k=2)
        whhzT = view(blob, o, "whhzT", 1536, "p (k m) -> p k m", k=2)
        woutT = view(blob, o, "woutT", 256, "p (k t) -> p k t", k=2)
        ident = view(blob, o, "ident", 128)
        brz_x = view(blob, o, "brz_x", 4)
        bhhn_x = view(blob, o, "bhhn_x", 2)
        nbihn_x = view(blob, o, "nbihn_x", 2)
        brz_z = view(blob, o, "brz_z", 4)
        bhhn_z = view(blob, o, "bhhn_z", 2)
        nbihn_z = view(blob, o, "nbihn_z", 2)
        ones = view(blob, o, "ones", 1)
        boutt = blob[0:1, o["bout"] : o["bout"] + 1]

        whhxT16 = view(blob16, o16, "whhxT", 1536, "p (k m) -> p k m", k=2)
        wihzT16 = view(blob16, o16, "wihzT", 1536, "p (k m) -> p k m", k=2)
        whhzT16 = view(blob16, o16, "whhzT", 1536, "p (k m) -> p k m", k=2)
        ident16 = view(blob16, o16, "ident", 128)
        wihxT16 = blob16[0:64, o16["wihxT"] : o16["wihxT"] + 3 * H]

        # ---- persistent state ----
        # fp16 trajectory buffers: zero pad slots 0..1, trajectory at 2..129
        # (keeps both the shifted matmul read [1:129] legal and the fp16 DVE
        # slices 4-byte aligned).  fp32 buffers: zero slot 0, traj at 1..128.
        xT = state.tile([64, BT], F32)
        xT16 = state.tile([64, BT], F16)
        Hx16 = state.tile([128, 2, BC, T + 2], F16)
        Hz16 = state.tile([128, 2, BC, T + 2], F16)
        Hx = state.tile([128, 2, BC, T + 1], F32)
        Hz = state.tile([128, 2, BC, T + 1], F32)
        Z16 = state.tile([128, 2, BC, T], F16)
        z016 = state.tile([128, 2, BC, T], F16)
        Z = state.tile([128, 2, BC, T], F32)
        rsb = state.tile([128, 2, BT], F32)
        zgsb = state.tile([128, 2, BT], F32)
        rsb16 = state.tile([128, 2, BT], F16)
        zgsb16 = state.tile([128, 2, BT], F16)

        nc.vector.memset(Hx16, 0.0)
        nc.vector.memset(Hz16, 0.0)
        nc.vector.memset(Hx, 0.0)
        nc.vector.memset(Hz, 0.0)

        # ---- transpose x -> xT [64, (b,t)] ----
        for ch in range(BT // 128):
            pst = psrz.tile([64, 128], F32, tag="prz")
            nc.tensor.transpose(out=pst, in_=xall[:, ch, :], identity=ident)
            nc.vector.tensor_copy(out=xT[:, ts(ch, 128)], in_=pst)

        nc.scalar.activation(out=xT16, in_=xT, func=AF.Copy)

        # ================= sweep bodies =================
        def sweep_lo(first, Hbuf16, Zrhs16, whhT16, brz, bhhn, nbihn, phase):
            """fp16 sweep.  Gates from lagged trajectory, affine scan, all
            matmul operands fp16 (PSUM accumulation stays fp32)."""
            for m in range(4):
                prz = psrz.tile([128, BT], F32, tag="prz")
                for nch in range(2):
                    ops = ([("wih", k) for k in range(2)] if phase == "z"
                           else [("wihx", 0)])
                    if not first:
                        ops += [("whh", k) for k in range(2)]
                    for i, (kind, k) in enumerate(ops):
                        st, sp = i == 0, i == len(ops) - 1
                        if kind == "wih":
                            nc.tensor.matmul(
                                prz[:, ts(nch, 512)], wihzT16[:, k, ts(m, 128)],
                                Zrhs16[:, k, ts(nch, 4), :], start=st, stop=sp)
                        elif kind == "wihx":
                            nc.tensor.matmul(
                                prz[:, ts(nch, 512)], wihxT16[:, ts(m, 128)],
                                xT16[:, ts(nch, 512)], start=st, stop=sp)
                        else:
                            nc.tensor.matmul(
                                prz[:, ts(nch, 512)], whhT16[:, k, ts(m, 128)],
                                Hbuf16[:, k, ts(nch, 4), 1 : T + 1],
                                start=st, stop=sp)
                dest = rsb16 if m < 2 else zgsb16
                nc.scalar.activation(
                    out=dest[:, m % 2, :], in_=prz, func=AF.Sigmoid,
                    bias=brz[:, m : m + 1], scale=1.0)

            d1 = scanw.tile([128, 2, BT], F16, tag="d1l")
            for km in range(2):
                m = 4 + km
                pg = psgi.tile([128, BT], F32, tag="pgi")
                for nch in range(2):
                    if phase == "z":
                        for k in range(2):
                            nc.tensor.matmul(
                                pg[:, ts(nch, 512)], wihzT16[:, k, ts(m, 128)],
                                Zrhs16[:, k, ts(nch, 4), :],
                                start=k == 0, stop=k == 1)
                    else:
                        nc.tensor.matmul(
                            pg[:, ts(nch, 512)], wihxT16[:, ts(m, 128)],
                            xT16[:, ts(nch, 512)], start=True, stop=True)
                gi_n = pg
                tmp = work.tile([128, BT], F16, tag="tmpl")
                if first:
                    nc.vector.tensor_scalar_mul(
                        out=tmp, in0=rsb16[:, km, :], scalar1=bhhn[:, km : km + 1])
                else:
                    pn = psn.tile([128, BT], F32, tag="pn")
                    for nch in range(2):
                        for k in range(2):
                            nc.tensor.matmul(
                                pn[:, ts(nch, 512)], whhT16[:, k, ts(m, 128)],
                                Hbuf16[:, k, ts(nch, 4), 1 : T + 1],
                                start=k == 0, stop=k == 1)
                    nc.vector.scalar_tensor_tensor(
                        out=tmp, in0=pn, scalar=bhhn[:, km : km + 1],
                        in1=rsb16[:, km, :], op0=OP.add, op1=OP.mult)
                npre = work.tile([128, BT], F16, tag="nprel")
                nc.vector.tensor_add(out=npre, in0=tmp, in1=gi_n)
                negn = work.tile([128, BT], F16, tag="negnl")
                nc.scalar.activation(
                    out=negn, in_=npre, func=AF.Tanh,
                    bias=nbihn[:, km : km + 1], scale=-1.0)
                nc.vector.scalar_tensor_tensor(
                    out=d1[:, km, :], in0=zgsb16[:, km, :], scalar=1.0,
                    in1=negn, op0=OP.subtract, op1=OP.mult)
            for km in range(2):
                for b in range(BC):
                    nc.vector.tensor_tensor_scan(
                        out=Hbuf16[:, km, b, 2 : T + 2],
                        data0=zgsb16[:, km, ts(b, T)],
                        data1=d1[:, km, ts(b, T)],
                        initial=0.0, op0=OP.mult, op1=OP.add)
            if phase == "z":
                for km in range(2):
                    zpre = work.tile([128, BC, T], F16, tag="zprel")
                    nc.vector.tensor_add(
                        out=zpre, in0=Hbuf16[:, km, :, 2 : T + 2],
                        in1=z016[:, km, :, :])
                    nc.scalar.activation(
                        out=Z16[:, km, :, :], in_=zpre, func=AF.Tanh)

        def sweep_hi(Hbuf, whhT, brz, bhhn, nbihn, phase):
            """fp32 polish sweep (never the first sweep of a phase)."""
            for m in range(4):
                prz = psrz.tile([128, BT], F32, tag="prz")
                for nch in range(2):
                    ops = ([("wih", k) for k in range(2)] if phase == "z"
                           else [("wihx", 0)])
                    ops += [("whh", k) for k in range(2)]
                    for i, (kind, k) in enumerate(ops):
                        st, sp = i == 0, i == len(ops) - 1
                        if kind == "wih":
                            nc.tensor.matmul(
                                prz[:, ts(nch, 512)], wihzT[:, k, ts(m, 128)],
                                Z[:, k, ts(nch, 4), :], start=st, stop=sp)
                        elif kind == "wihx":
                            nc.tensor.matmul(
                                prz[:, ts(nch, 512)], wihxT[:, ts(m, 128)],
                                xT[:, ts(nch, 512)], start=st, stop=sp)
                        else:
                            nc.tensor.matmul(
                                prz[:, ts(nch, 512)], whhT[:, k, ts(m, 128)],
                                Hbuf[:, k, ts(nch, 4), 0:T], start=st, stop=sp)
                dest = rsb if m < 2 else zgsb
                nc.scalar.activation(
                    out=dest[:, m % 2, :], in_=prz, func=AF.Sigmoid,
                    bias=brz[:, m : m + 1], scale=1.0)

            d1 = scanw.tile([128, 2, BT], F32, tag="d1h")
            for km in range(2):
                m = 4 + km
                pg = psgi.tile([128, BT], F32, tag="pgi")
                for nch in range(2):
                    if phase == "z":
                        for k in range(2):
                            nc.tensor.matmul(
                                pg[:, ts(nch, 512)], wihzT[:, k, ts(m, 128)],
                                Z[:, k, ts(nch, 4), :], start=k == 0, stop=k == 1)
                    else:
                        nc.tensor.matmul(
                            pg[:, ts(nch, 512)], wihxT[:, ts(m, 128)],
                            xT[:, ts(nch, 512)], start=True, stop=True)
                gi_n = pg
                pn = psn.tile([128, BT], F32, tag="pn")
                for nch in range(2):
                    for k in range(2):
                        nc.tensor.matmul(
                            pn[:, ts(nch, 512)], whhT[:, k, ts(m, 128)],
                            Hbuf[:, k, ts(nch, 4), 0:T], start=k == 0, stop=k == 1)
                tmp = work.tile([128, BT], F32, tag="tmph")
                nc.vector.scalar_tensor_tensor(
                    out=tmp, in0=pn, scalar=bhhn[:, km : km + 1],
                    in1=rsb[:, km, :], op0=OP.add, op1=OP.mult)
                npre = work.tile([128, BT], F32, tag="npreh")
                nc.vector.tensor_add(out=npre, in0=tmp, in1=gi_n)
                negn = work.tile([128, BT], F32, tag="negnh")
                nc.scalar.activation(
                    out=negn, in_=npre, func=AF.Tanh,
                    bias=nbihn[:, km : km + 1], scale=-1.0)
                nc.vector.scalar_tensor_tensor(
                    out=d1[:, km, :], in0=zgsb[:, km, :], scalar=1.0,
                    in1=negn, op0=OP.subtract, op1=OP.mult)
            for km in range(2):
                for b in range(BC):
                    nc.vector.tensor_tensor_scan(
                        out=Hbuf[:, km, b, 1 : T + 1],
                        data0=zgsb[:, km, ts(b, T)],
                        data1=d1[:, km, ts(b, T)],
                        initial=0.0, op0=OP.mult, op1=OP.add)
            if phase == "z":
                for km in range(2):
                    zpre = work.tile([128, BC, T], F32, tag="zpreh")
                    nc.vector.tensor_add(
                        out=zpre, in0=Hbuf[:, km, :, 1 : T + 1],
                        in1=Hx[:, km, :, 1 : T + 1])
                    nc.scalar.activation(
                        out=Z[:, km, :, :], in_=zpre, func=AF.Tanh)

        # ================= x phase =================
        for s in range(NSX_LO):
            sweep_lo(s == 0, Hx16, None, whhxT16, brz_x, bhhn_x, nbihn_x, "x")
        # fp16 -> fp32 trajectory handoff
        nc.vector.tensor_copy(
            out=Hx[:, :, :, 1 : T + 1], in_=Hx16[:, :, :, 2 : T + 2])
        for s in range(NSX_HI):
            sweep_hi(Hx, whhxT, brz_x, bhhn_x, nbihn_x, "x")

        # z0 handoff: z016 = Z16_init = fp16 cast of the converged x trajectory
        nc.scalar.activation(
            out=z016[:, :, :, :], in_=Hx[:, :, :, 1 : T + 1], func=AF.Copy)
        nc.vector.tensor_copy(out=Z16[:, :, :, :], in_=z016)

        # ================= z phase =================
        for s in range(NSZ_LO):
            sweep_lo(s == 0, Hz16, Z16, whhzT16, brz_z, bhhn_z, nbihn_z, "z")
        nc.vector.tensor_copy(
            out=Hz[:, :, :, 1 : T + 1], in_=Hz16[:, :, :, 2 : T + 2])
        nc.vector.tensor_copy(out=Z[:, :, :, :], in_=Z16)
        for s in range(NSZ_HI):
            sweep_hi(Hz, whhzT, brz_z, bhhn_z, nbihn_z, "z")

        # ================= head =================
        red = work.tile([128, 2, BC], F32, tag="red")
        for k in range(2):
            prod = work.tile([128, BC, T], F32, tag="prod")
            wslice = woutT[:, k, :]
            wbcast = bass.AP(
                tensor=wslice.tensor, offset=wslice.offset,
                ap=[wslice.ap[0], [0, BC], wslice.ap[1]])
            nc.vector.tensor_mul(out=prod, in0=Z[:, k, :, :], in1=wbcast)
            nc.vector.reduce_sum(
                out=red[:, k, :], in_=prod, axis=mybir.AxisListType.X)
        redk = work.tile([128, BC], F32, tag="redk")
        nc.vector.tensor_add(out=redk, in0=red[:, 0, :], in1=red[:, 1, :])
        ph = psn.tile([1, BC], F32, tag="pn")
        nc.tensor.matmul(ph, ones, redk, start=True, stop=True)
        osb = work.tile([1, BC], F32, tag="osb")
        nc.vector.tensor_scalar_add(out=osb, in0=ph, scalar1=boutt[0:1, 0:1])
        nc.sync.dma_start(out=out_d[:, :], in_=osb)

    nc.compile()
    return nc


def _prep_shared(inputs):
    f = np.float32
    Wih_x, Whh_x = inputs["Wih_x"], inputs["Whh_x"]
    Wih_z, Whh_z = inputs["Wih_z"], inputs["Whh_z"]
    bih_x, bhh_x = inputs["bih_x"], inputs["bhh_x"]
    bih_z, bhh_z = inputs["bih_z"], inputs["bhh_z"]

    def biases(bih, bhh):
        brz = (bih[: 2 * H] + bhh[: 2 * H]).astype(f).reshape(4, 128).T.copy()
        bhhn = bhh[2 * H :].astype(f).reshape(2, 128).T.copy()
        nbihn = (-bih[2 * H :]).astype(f).reshape(2, 128).T.copy()
        return brz, bhhn, nbihn

    brz_x, bhhn_x, nbihn_x = biases(bih_x, bhh_x)
    brz_z, bhhn_z, nbihn_z = biases(bih_z, bhh_z)
    woutT = inputs["Wout"].astype(f).reshape(T, 2, 128).transpose(2, 1, 0)

    def packT(w):  # [768, 256] -> [128, (k,m)=1536] transposed tiles
        return np.ascontiguousarray(
            w.T.astype(f).reshape(2, 128, 3 * H).transpose(1, 0, 2).reshape(128, -1)
        )

    o = _blob_offsets()
    blob = np.zeros((128, BLOB_COLS), dtype=f)

    def put(name, arr):
        arr = np.asarray(arr, dtype=f)
        blob[: arr.shape[0], o[name] : o[name] + arr.shape[1]] = arr

    put("whhxT", packT(Whh_x))
    put("wihzT", packT(Wih_z))
    put("whhzT", packT(Whh_z))
    put("woutT", woutT.reshape(128, 256))
    put("ident", np.eye(128, dtype=f))
    put("brz_x", brz_x)
    put("bhhn_x", bhhn_x)
    put("nbihn_x", nbihn_x)
    put("brz_z", brz_z)
    put("bhhn_z", bhhn_z)
    put("nbihn_z", nbihn_z)
    put("ones", np.ones((128, 1), dtype=f))
    put("bout", inputs["bout"].astype(f).reshape(1, 1))
    put("wihxT", Wih_x.T.astype(f))

    o16 = _blob16_offsets()
    blob16 = np.zeros((128, BLOB16_COLS), dtype=np.float16)

    def put16(name, arr):
        arr = np.asarray(arr, dtype=np.float16)
        blob16[: arr.shape[0], o16[name] : o16[name] + arr.shape[1]] = arr

    put16("whhxT", packT(Whh_x))
    put16("wihzT", packT(Wih_z))
    put16("whhzT", packT(Whh_z))
    put16("ident", np.eye(128, dtype=np.float16))
    put16("wihxT", Wih_x.T.astype(np.float16))
    return blob, blob16


def kernel(**inputs):
    from concourse.bass_utils import run_bass_kernel_spmd

    if "nc" not in _cache:
        _cache["nc"] = _build_program()
    nc = _cache["nc"]

    blob, blob16 = _prep_shared(inputs)
    o = _blob_offsets()
    x = inputs["x"].astype(np.float32)
    in_maps = []
    for c in range(NCORES):
        cb = blob.copy()
        xs = (
            x[c * BC : (c + 1) * BC]
            .reshape(BT // 128, 128, D)
            .transpose(1, 0, 2)
            .reshape(128, -1)
        )
        cb[:, o["x"] : o["x"] + xs.shape[1]] = xs
        in_maps.append({"blob": cb, "blob16": blob16})

    import contextlib
    import os

    prof_dir = os.environ.get("KERNEL_PROFILE_DIR")
    prof_ctx = contextlib.nullcontext()
    if prof_dir:
        try:
            from antenv.axon_hooks import get_axon_ntff_profile_hook

            hook = get_axon_ntff_profile_hook()
            if hook is not None:
                prof_ctx = hook(prof_dir, None)
        except ImportError:
            pass
    with prof_ctx:
        res = run_bass_kernel_spmd(nc, in_maps, core_ids=list(range(NCORES)))
    kernel._last_results = res
    out = np.empty((B, 1), dtype=np.float32)
    for c in range(NCORES):
        out[c * BC : (c + 1) * BC, 0] = res.results[c]["out"][0]
    return out
